# revision 2
# baseline (speedup 1.0000x reference)
"""CQT magnitude kernel for Trainium2 (8 NeuronCores, Bass/Tile).

Strategy (v2)
-------------
C[k, n] = sum_l xpad[n*HOP + l] * kernel[k, l], regrouped over 128-wide
l-chunks: with X128[p, j] = xpad[j*128 + p] and HOP = 512 = 4*128,

    C[k, n] = sum_c sum_p kernel[k, c*128 + p] * X128[p, c + 4n]

One PE matmul per (l-chunk, re/im): lhsT = kernelT chunk ([128 l, M bins]),
rhs = strided view of X128, accumulated in PSUM across chunks.

v2 refinements over the 95us baseline (all exact, no numerics change):
 * M-trimmed weight loads: constant-Q support shrinks with frequency, so
   chunk c only has M_c active bins (a prefix, since bins are sorted by
   support).  LDWEIGHTS cost scales with weight *columns*, so loading
   [128, M_c] instead of [128, 128] cuts total LDWEIGHTS time ~2.5x.
 * Frame-trimmed matmuls: frames near the signal edges read all-zero
   padding chunks; per-slot the rhs is restricted to the valid frame
   window, cutting streamed matmul columns ~25%.
 * Strided chunk assignment (core q gets chunks 8s+q) keeps the frame
   windows SPMD-uniform (program identical across cores; all per-core
   variation lives in the packed inputs).
 * 3 kt DMA groups + 1 xi DMA + 4 output DMAs (vs 19 DMAs): fewer
   cross-engine semaphores shrink the fixed init/teardown storms.
 * PSUM init via two untrimmed full-coverage start=True slots per bin
   half; everything else accumulates with start=False.

Numerics: operands bf16 (PE streams 1 col/cycle), f32 PSUM accumulation;
host sums the 8 per-core partials and takes sqrt(re^2 + im^2).
"""

import numpy as np

# ---- problem constants (hardcoded per contract) ----
SR = 44100
BPO = 36
KBINS = 252
FMIN = 32.70319566257483
QF = 1.0 / (2.0 ** (1.0 / BPO) - 1.0)
SR_B, SR_TR, SR_T = 2, 2, 65536
NTRACKS = SR_B * SR_TR            # 4
L = 69376                          # filterbank window length
HOP = 512
PCH = 128
NCH = L // PCH                     # 542 l-chunks
NF = 1 + SR_T // HOP               # 129 frames
NCORES = 8
M1C0 = 247                         # first m1 chunk
NS0 = 68                           # m0 slots per core (542/8 rounded up)
NS1 = 6                           # m1 slots per core (48/8)
INIT0 = 33                         # untrimmed m0 init slot (center chunks)
J_VALID_LO, J_VALID_HI = 271, 782  # nonzero xpad chunk range (inclusive)
XPAD_CH = 1056
NWARM = 12

# ---- derived slot tables (exact sparsity of the constant-Q bank) ----


def _build_tables():
    freqs = FMIN * 2.0 ** (np.arange(KBINS) / BPO)
    lens = QF * SR / freqs
    lo = np.floor((L // 2 - lens / 2) / PCH).astype(int)
    hi = np.ceil((L // 2 + lens / 2) / PCH).astype(int)
    m0c = np.zeros(NCH + 8, int)
    m1c = np.zeros(NCH + 8, int)
    for k in range(128):
        m0c[lo[k] : hi[k]] = np.maximum(m0c[lo[k] : hi[k]], k + 1)
    for k in range(128, KBINS):
        m1c[lo[k] : hi[k]] = np.maximum(m1c[lo[k] : hi[k]], k - 127)
    m0s = np.array([max(m0c[8 * s + q] for q in range(8)) for s in range(NS0)])
    m1s = np.array(
        [max(m1c[M1C0 + 8 * s + q] for q in range(8)) for s in range(NS1)]
    )
    m0s = (m0s + 1) // 2 * 2
    m1s = (m1s + 1) // 2 * 2
    m0s[INIT0] = 128   # init slots cover every bin row (PSUM has_written)
    m1s[0] = 128

    def nrng(cl, ch):
        return max(0, -(-(J_VALID_LO - ch) // 4)), min(
            NF - 1, (J_VALID_HI - cl) // 4
        )

    f0 = [nrng(8 * s, 8 * s + 7) for s in range(NS0)]
    f1 = [nrng(M1C0 + 8 * s, M1C0 + 8 * s + 7) for s in range(NS1)]
    f0[INIT0] = (0, NF - 1)   # init slots untrimmed (full column coverage)
    f1[0] = (0, NF - 1)

    # emission order: m0 init (re, im), m0-re, m0-im, m1 init, m1-re, m1-im
    entries = []  # (m, s, part, M, n0, n1, start, stop)
    entries.append((0, INIT0, 0, 128, 0, NF - 1, True, False))
    entries.append((0, INIT0, 1, 128, 0, NF - 1, True, False))
    for part in range(2):
        for s in range(NS0):
            if s == INIT0:
                continue
            last = s == NS0 - 1
            entries.append(
                (0, s, part, int(m0s[s]), f0[s][0], f0[s][1], False, last)
            )
    entries.append((1, 0, 0, 128, 0, NF - 1, True, False))
    entries.append((1, 0, 1, 128, 0, NF - 1, True, False))
    for part in range(2):
        for s in range(1, NS1):
            last = s == NS1 - 1
            entries.append(
                (1, s, part, int(m1s[s]), f1[s][0], f1[s][1], False, last)
            )

    # kt column offsets + 3 DMA group split (~equal bytes after group 0)
    offs = np.cumsum([0] + [e[3] for e in entries])
    total = int(offs[-1])
    g0_end = 26                       # inits + the cheap small-M head
    rest = total - int(offs[g0_end])
    t1 = int(offs[g0_end]) + rest // 2
    g1_end = int(np.searchsorted(offs, t1))
    gsplits = [0, g0_end, g1_end, len(entries)]
    umax = 0
    for m, s, part, M, n0, n1, _, _ in entries:
        u0 = 8 * s + 4 * n0 - (M1C0 if m == 0 else 0)
        umax = max(umax, u0 + 4 * (n1 - n0))
    return entries, offs, gsplits, umax + 1


_ENTRIES, _KTOFF, _GSPL, _XIU = _build_tables()
_GCOLS = [int(_KTOFF[_GSPL[g + 1]] - _KTOFF[_GSPL[g]]) for g in range(3)]

_PROG = None


def _build_program():
    import concourse.bass as bass
    import concourse.mybir as mybir
    from concourse import bacc
    from concourse.tile import TileContext

    f32 = mybir.dt.float32
    bf16 = mybir.dt.bfloat16

    nc = bacc.Bacc(None, name="cqt_spmd2")
    kt_d = [
        nc.dram_tensor(f"kt{g}", [128, _GCOLS[g]], bf16, kind="ExternalInput")
        for g in range(3)
    ]
    xi_d = nc.dram_tensor("xi", [128, _XIU, 4], bf16, kind="ExternalInput")
    out_d = nc.dram_tensor("out", [128, 8 * 2 * NF], f32, kind="ExternalOutput")

    with TileContext(nc) as tc:
        with (
            tc.tile_pool(name="xip", bufs=1) as xip,
            tc.tile_pool(name="ktp", bufs=3) as ktp,
            tc.tile_pool(name="wp", bufs=1) as wp,
            tc.tile_pool(name="accp", bufs=1, space="PSUM") as accp,
        ):
            kt_t = []
            for g in range(3):
                t = ktp.tile([128, _GCOLS[g]], bf16, tag=f"kt{g}", name=f"kt{g}")
                nc.gpsimd.dma_start(out=t, in_=kt_d[g][:, :])
                kt_t.append(t)
                if g == 0:
                    xi_t = xip.tile([128, _XIU, 4], bf16)
                    nc.gpsimd.dma_start(out=xi_t, in_=xi_d[:, :, :])

            accs = [
                accp.tile([128, 2 * NF], f32, tag=f"acc{b}", name=f"acc{b}")
                for b in range(8)
            ]

            # PE pre-warm while the first DMAs land (HAM clock-gate opens
            # after ~3.4us of sustained PE activity).  Garbage lands in
            # bank 7, re-initialized by the m1-im start=True matmul later.
            wtile = wp.tile([128, 128], bf16)
            nc.vector.memset(wtile, 0.0)
            for _ in range(NWARM):
                nc.tensor.matmul(
                    accs[7][:, :128], wtile, wtile, start=True, stop=True
                )

            st = wp.tile([128, 8 * 2 * NF], f32, tag="st", name="st")

            g = 0
            flushed = 0
            probed = False
            for ei, (m, s, part, M, n0, n1, first, last) in enumerate(_ENTRIES):
                while ei >= _GSPL[g + 1]:
                    g += 1
                if ei == 60 and not probed:
                    # --- rate probes (mid-solid): contiguous vs strided rhs ---
                    probed = True
                    for rep in range(10):
                        rhs = bass.AP(
                            tensor=kt_t[1].tensor,
                            offset=kt_t[1].offset,
                            ap=[kt_t[1].ap[0], [1, 256]],
                        )
                        nc.tensor.matmul(
                            accs[7][:, :256], kt_t[1][:, 300:428],
                            rhs, start=True, stop=True,
                        )
                    for rep in range(10):
                        rhs = bass.AP(
                            tensor=kt_t[1].tensor,
                            offset=kt_t[1].offset,
                            ap=[kt_t[1].ap[0], [16, 128], [1, 2]],
                        )
                        nc.tensor.matmul(
                            accs[7][:, :256], kt_t[1][:, 300:428],
                            rhs, start=True, stop=True,
                        )
                off = int(_KTOFF[ei] - _KTOFF[_GSPL[g]])
                lhsT = kt_t[g][:, off : off + M]
                u0 = 8 * s + 4 * n0 - (M1C0 if m == 0 else 0)
                F = n1 - n0 + 1
                for tp in range(2):
                    rhs = bass.AP(
                        tensor=xi_t.tensor,
                        offset=xi_t.offset + u0 * 4 + tp * 2,
                        ap=[xi_t.ap[0], [16, F], [1, 2]],
                    )
                    b = m * 4 + part * 2 + tp
                    out = accs[b][:M, 2 * n0 : 2 * n1 + 2]
                    nc.tensor.matmul(out, lhsT, rhs, start=first, stop=last)
                if last:
                    # this (m, part) pass is done: flush its two banks and,
                    # per pair, DMA the staged half out while later passes
                    # still stream on the PE
                    for tp in range(2):
                        b = m * 4 + part * 2 + tp
                        nc.vector.tensor_copy(
                            st[:, b * 2 * NF : (b + 1) * 2 * NF], accs[b]
                        )
                    lo = flushed * 2 * NF
                    hi = (flushed + 2) * 2 * NF
                    nc.gpsimd.dma_start(out=out_d[:, lo:hi], in_=st[:, lo:hi])
                    flushed += 2
    nc.finalize()
    _dedupe_ldweights(nc)
    return nc


def _dedupe_ldweights(nc):
    """Drop back-to-back InstLdweights with identical weights APs.

    The legalizer emits one LDWEIGHTS per MATMUL; consecutive matmuls that
    share lhsT (the two track-pair matmuls of each slot-part) reload the
    same weights.  LDWEIGHTS is a fixed ~115ns serialized on the PE weight
    path, so each removed reload is straight wall-clock.  Only duplicates
    carrying no semaphore waits/updates are dropped.
    """
    for fn in nc.m.functions:
        for bb in fn.blocks:
            insts = list(bb.instructions)
            keep = []
            prev_key = None
            for inst in insts:
                if type(inst).__name__ == 'InstLdweights':
                    key = str(inst.ins[0])
                    si = inst.sync_info
                    clean = not si or (
                        len(si.on_wait) == 0 and len(si.on_update) == 0
                    )
                    if key == prev_key and clean:
                        continue
                    prev_key = key
                keep.append(inst)
            if len(keep) != len(insts):
                bb.instructions = keep


def _pack_inputs(x, kr, ki):
    import ml_dtypes

    bf16 = ml_dtypes.bfloat16
    xf = np.ascontiguousarray(
        np.asarray(x, dtype=np.float32).reshape(NTRACKS, SR_T)
    )
    kr = np.asarray(kr, dtype=np.float32)
    ki = np.asarray(ki, dtype=np.float32)

    krT0 = np.ascontiguousarray(kr[:128].T)   # [L, 128]
    kiT0 = np.ascontiguousarray(ki[:128].T)

    def padT(mat):
        buf = np.zeros((128, L), np.float32)
        buf[: mat.shape[0]] = mat
        return np.ascontiguousarray(buf.T)

    krT1 = padT(kr[128:])
    kiT1 = padT(ki[128:])
    halves = {(0, 0): krT0, (0, 1): kiT0, (1, 0): krT1, (1, 1): kiT1}

    xpad = np.zeros((NTRACKS, XPAD_CH * PCH), np.float32)
    xpad[:, L // 2 : L // 2 + SR_T] = xf
    XI_full = np.ascontiguousarray(
        xpad.reshape(NTRACKS, XPAD_CH, PCH).transpose(2, 1, 0)
    )  # [128, 1056, 4]

    in_maps = []
    for q in range(NCORES):
        kt = np.zeros((128, int(_KTOFF[-1])), np.float32)
        for ei, (m, s, part, M, n0, n1, _, _) in enumerate(_ENTRIES):
            c = 8 * s + q + (M1C0 if m == 1 else 0)
            if c >= NCH + (M1C0 if m == 1 else 0) and m == 0:
                continue  # chunks 542/543 on cores 6-7: zero weights
            off = int(_KTOFF[ei])
            kt[:, off : off + M] = halves[(m, part)][c * 128 : (c + 1) * 128, :M]
        ktb = kt.astype(bf16)
        im = {
            f"kt{g}": np.ascontiguousarray(
                ktb[:, int(_KTOFF[_GSPL[g]]) : int(_KTOFF[_GSPL[g + 1]])]
            )
            for g in range(3)
        }
        im["xi"] = np.ascontiguousarray(
            XI_full[:, M1C0 + q : M1C0 + q + _XIU].astype(bf16)
        )
        in_maps.append(im)
    return in_maps


def _combine(outs):
    re_acc = np.zeros((KBINS, NTRACKS, NF), np.float32)
    im_acc = np.zeros((KBINS, NTRACKS, NF), np.float32)
    for q in range(NCORES):
        o = outs[q].reshape(128, 8, NF, 2)
        for b in range(8):
            m, part, tp = b >> 2, (b >> 1) & 1, b & 1
            rows = slice(0, 128) if m == 0 else slice(128, KBINS)
            nrows = 128 if m == 0 else KBINS - 128
            tgt = re_acc if part == 0 else im_acc
            tgt[rows, tp * 2 : (tp + 1) * 2] += o[:nrows, b].transpose(0, 2, 1)
    y = np.sqrt(re_acc**2 + im_acc**2)  # [252, 4, 129]
    return np.ascontiguousarray(
        y.reshape(KBINS, SR_B, SR_TR, NF).transpose(1, 0, 3, 2)
    )


def kernel(x, kr, ki):
    global _PROG
    from concourse.bass_utils import run_bass_kernel_spmd

    if _PROG is None:
        _PROG = _build_program()
    in_maps = _pack_inputs(x, kr, ki)
    res = run_bass_kernel_spmd(_PROG, in_maps, core_ids=list(range(NCORES)))
    outs = [res.results[q]["out"] for q in range(NCORES)]
    return _combine(outs)



# revision 6
# speedup vs baseline: 2.0011x; 2.0011x over previous
"""CQT magnitude kernel for Trainium2 (8 NeuronCores, Bass/Tile).

Strategy (v3)
-------------
C[k, n] = sum_l xpad[n*HOP + l] * kernel[k, l], tiled over 128-wide
l-chunks.  Core q owns chunks c = 8s+q (m0 = bins 0-127) and
c = 247+8s+q (m1 = bins 128-251); host sums the 8 per-core partials.

Key points vs v2 (74.8us -> target ~31us):
 * Contiguous rhs streams.  Core q only ever touches xpad chunks of a
   single residue class mod 4 (j = q mod 4 for m0, q+3 mod 4 for m1), so
   xi is packed per-class as [128, v, 4 tracks] with v = signal chunk/4.
   The matmul rhs walk [ [4,F], [1,4] ] is then fully sequential in
   SBUF.  Measured: strided rhs streams at 0.90 ns/col, contiguous at
   0.427 ns/col (1 col/cycle @ 2.4 GHz) -- a 2.1x PE speedup.
 * re/im interleaved weight columns: col 2t = kr[bin t], 2t+1 = ki.
   For chunks with M<=64 active bins one matmul computes both parts
   (row 2t = re, 2t+1 = im), halving streamed columns there.  Bins are
   split lo (0-63) / hi (64-127) per half so PSUM row meanings stay
   uniform across chunks; hi entries exist only where M>64.
 * 8 PSUM accumulators: (m0,m1) x (lo,hi) x (frame half 0-64 / 65-128),
   each [128, <=260] f32 (2KB bank limit forces the frame split).
 * Banks are initialized by zero-weight matmuls (start=True, full
   rectangle) during the DMA wait -- they double as PE p-state warmup.
   All real entries accumulate with start=False in any order, so the
   short, LDWEIGHTS-bound edge entries run first while the PE clock
   still ramps.
 * Flush pipelining: m1 pass runs early and its banks flush under the
   long m0 pass; m0-hi banks flush under the m0 lo-only tail; only the
   m0-lo flush remains in the tail.  Output staged to bf16 (halves
   DMA; partial sums re-summed on host in f32, ~0.1% error).
"""

import numpy as np

# ---- problem constants (hardcoded per contract) ----
SR = 44100
BPO = 36
KBINS = 252
FMIN = 32.70319566257483
QF = 1.0 / (2.0 ** (1.0 / BPO) - 1.0)
SR_B, SR_TR, SR_T = 2, 2, 65536
NTRACKS = SR_B * SR_TR            # 4
L = 69376                          # filterbank window length
HOP = 512
PCH = 128
NCH = L // PCH                     # 542 l-chunks
NF = 1 + SR_T // HOP               # 129 frames
NCORES = 8
M1C0 = 247                         # first m1 chunk
NS0 = 68                           # m0 slots per core
NS1 = 6                            # m1 slots per core (48 chunks)
J_VALID_LO, J_VALID_HI = 271, 782  # nonzero xpad chunk range (inclusive)
XPAD_CH = 1056
FH = 65                            # frame-half boundary: fh0=[0,64], fh1=[65,128]
VA0, VA = 64, 134                  # xiA v-window (v = (j - q)/4)
VB0, VB = 64, 133                  # xiB v-window (v = (j - q - 3)/4)
N_FILL = 14                        # PE warmup filler matmuls

# ---- slot tables ----


def _build_tables():
    freqs = FMIN * 2.0 ** (np.arange(KBINS) / BPO)
    lens = QF * SR / freqs
    lo = np.floor((L // 2 - lens / 2) / PCH).astype(int)
    hi = np.ceil((L // 2 + lens / 2) / PCH).astype(int)
    m0c = np.zeros(NCH + 8, int)
    m1c = np.zeros(NCH + 8, int)
    for k in range(128):
        m0c[lo[k] : hi[k]] = np.maximum(m0c[lo[k] : hi[k]], k + 1)
    for k in range(128, KBINS):
        m1c[lo[k] : hi[k]] = np.maximum(m1c[lo[k] : hi[k]], k - 127)
    m0s = [max(m0c[8 * s + q] for q in range(8)) for s in range(NS0)]
    m1s = [max(m1c[M1C0 + 8 * s + q] for q in range(8)) for s in range(NS1)]

    def nrng(cl, ch):
        return max(0, -(-(J_VALID_LO - ch) // 4)), min(
            NF - 1, (J_VALID_HI - cl) // 4
        )

    f0 = [nrng(8 * s, 8 * s + 7) for s in range(NS0)]
    f1 = [nrng(M1C0 + 8 * s, M1C0 + 8 * s + 7) for s in range(NS1)]

    # emission order: phase E (ramp, short-F edge slots), phase B (m1),
    # phase A (m0 rest, hi-carrying slots first)
    e_slots = []
    for i in range(9):
        e_slots.append(i)
        e_slots.append(67 - i)
    hi_slots = [s for s in range(NS0) if m0s[s] > 64]
    a_rest = [s for s in range(9, 59) if s not in hi_slots]
    b_order = [2, 3, 1, 4, 0, 5]

    # entry: (m, s, kind, cols, n0, n1)   kind 0=lo 1=hi
    entries = []
    for s in e_slots:
        entries.append((0, s, 0, 2 * min(m0s[s], 64), f0[s][0], f0[s][1]))
    for s in b_order:
        entries.append((1, s, 0, 2 * min(m1s[s], 64), f1[s][0], f1[s][1]))
        if m1s[s] > 64:
            entries.append((1, s, 1, 2 * (m1s[s] - 64), f1[s][0], f1[s][1]))
    for s in hi_slots:
        entries.append((0, s, 0, 2 * min(m0s[s], 64), f0[s][0], f0[s][1]))
        entries.append((0, s, 1, 2 * (m0s[s] - 64), f0[s][0], f0[s][1]))
    for s in a_rest:
        entries.append((0, s, 0, 2 * min(m0s[s], 64), f0[s][0], f0[s][1]))

    n_e = len(e_slots)
    n_b = sum(1 + (m1s[s] > 64) for s in b_order)
    n_ahi = 2 * len(hi_slots)
    # kt DMA groups: E | B | A-hi+rest split in 3
    offs = np.cumsum([0] + [e[3] for e in entries])
    b0 = n_e
    a0 = n_e + n_b
    rest = int(offs[-1] - offs[a0])
    t1 = int(offs[a0]) + rest // 3
    t2 = int(offs[a0]) + 2 * rest // 3
    g2 = int(np.searchsorted(offs, t1))
    g3 = int(np.searchsorted(offs, t2))
    gsplits = [0, b0, a0, g2, g3, len(entries)]
    return entries, offs, gsplits, n_e + n_b + n_ahi


_ENTRIES, _KTOFF, _GSPL, _A_HI_END = _build_tables()
_NG = len(_GSPL) - 1
_GCOLS = [int(_KTOFF[_GSPL[g + 1]] - _KTOFF[_GSPL[g]]) for g in range(_NG)]

# bank ids: 0=A_lo0 1=A_lo1 2=A_hi0 3=A_hi1 4=B_lo0 5=B_lo1 6=B_hi0 7=B_hi1
_BANK_COLS = [260, 256, 260, 256, 260, 256, 260, 256]


def _bank_of(m, kind, fh):
    return (4 if m == 1 else 0) + 2 * kind + fh


def _fh_windows(n0, n1):
    """Split [n0, n1] at the frame-half boundary; yields (fh, a, b)."""
    out = []
    if n0 < FH:
        out.append((0, n0, min(n1, FH - 1)))
    if n1 >= FH:
        out.append((1, max(n0, FH), n1))
    return out


def _last_writer_per_bank():
    last = {}
    for ei, (m, s, kind, cols, n0, n1) in enumerate(_ENTRIES):
        for fh, a, b in _fh_windows(n0, n1):
            last[_bank_of(m, kind, fh)] = ei
    return last


_LAST = _last_writer_per_bank()

_PROG = None


def _build_program():
    import concourse.bass as bass
    import concourse.mybir as mybir
    from concourse import bacc
    from concourse.tile import TileContext

    f32 = mybir.dt.float32
    bf16 = mybir.dt.bfloat16

    nc = bacc.Bacc(None, name="cqt_spmd3")
    kt_d = [
        nc.dram_tensor(f"kt{g}", [128, _GCOLS[g]], bf16, kind="ExternalInput")
        for g in range(_NG)
    ]
    xa_d = nc.dram_tensor("xa", [128, VA, 4], bf16, kind="ExternalInput")
    xb_d = nc.dram_tensor("xb", [128, VB, 4], bf16, kind="ExternalInput")
    OUTC = sum(_BANK_COLS)
    out_d = nc.dram_tensor("out", [128, OUTC], bf16, kind="ExternalOutput")

    with TileContext(nc) as tc:
        with (
            tc.tile_pool(name="xp", bufs=1) as xp,
            tc.tile_pool(name="ktp", bufs=1) as ktp,
            tc.tile_pool(name="wp", bufs=1) as wp,
            tc.tile_pool(name="accp", bufs=1, space="PSUM") as accp,
        ):
            wtile = wp.tile([128, 128], bf16, tag="wt", name="wt")
            nc.vector.memset(wtile, 0.0)

            xa_t = xp.tile([128, VA, 4], bf16, tag="xa", name="xa")
            xb_t = xp.tile([128, VB, 4], bf16, tag="xb", name="xb")
            kt_t = [
                ktp.tile([128, _GCOLS[g]], bf16, tag=f"kt{g}", name=f"kt{g}")
                for g in range(_NG)
            ]
            # DMA issue split across engines; critical (xa, ktE) first
            nc.gpsimd.dma_start(out=xa_t, in_=xa_d[:, :, :])
            nc.scalar.dma_start(out=kt_t[0], in_=kt_d[0][:, :])
            nc.scalar.dma_start(out=xb_t, in_=xb_d[:, :, :])
            nc.scalar.dma_start(out=kt_t[1], in_=kt_d[1][:, :])
            nc.gpsimd.dma_start(out=kt_t[2], in_=kt_d[2][:, :])
            nc.gpsimd.dma_start(out=kt_t[3], in_=kt_d[3][:, :])
            nc.gpsimd.dma_start(out=kt_t[4], in_=kt_d[4][:, :])

            # full 2KB banks so no tile ever crosses a PSUM bank boundary
            accs = [
                accp.tile([128, 512], f32, tag=f"acc{b}", name=f"acc{b}")
                for b in range(8)
            ]

            # zero-weight bank inits (full-rect start=True) + p-state warmup
            def zrhs(cols):
                return bass.AP(
                    tensor=wtile.tensor,
                    offset=wtile.offset,
                    ap=[wtile.ap[0], [0, cols // 4], [1, 4]],
                )

            for b in range(8):
                nc.tensor.matmul(
                    accs[b][:128, : _BANK_COLS[b]], wtile[:, :128],
                    zrhs(_BANK_COLS[b]), start=True, stop=False,
                )
            for _ in range(N_FILL):
                nc.tensor.matmul(
                    accs[0][:128, :260], wtile[:, :128], zrhs(260),
                    start=False, stop=False,
                )

            st = wp.tile([128, OUTC], bf16, tag="st", name="st")
            boff = np.cumsum([0] + _BANK_COLS)

            def flush(banks, eng_cycle):
                for i, b in enumerate(banks):
                    eng = eng_cycle[i % len(eng_cycle)]
                    src = accs[b][:128, : _BANK_COLS[b]]
                    dst = st[:, int(boff[b]) : int(boff[b + 1])]
                    if hasattr(eng, "tensor_copy"):
                        eng.tensor_copy(dst, src)
                    else:
                        eng.copy(dst, src)
                lo = int(boff[banks[0]])
                hi = int(boff[banks[-1] + 1])
                nc.gpsimd.dma_start(out=out_d[:, lo:hi], in_=st[:, lo:hi])

            g = 0
            for ei, (m, s, kind, cols, n0, n1) in enumerate(_ENTRIES):
                while ei >= _GSPL[g + 1]:
                    g += 1
                off = int(_KTOFF[ei] - _KTOFF[_GSPL[g]])
                lhsT = kt_t[g][:, off : off + cols]
                xi = xb_t if m == 1 else xa_t
                vbase = (61 + 2 * s - VB0) if m == 1 else (2 * s - VA0)
                for fh, a, b in _fh_windows(n0, n1):
                    F = b - a + 1
                    rhs = bass.AP(
                        tensor=xi.tensor,
                        offset=xi.offset + (vbase + a) * 4,
                        ap=[xi.ap[0], [4, F], [1, 4]],
                    )
                    bk = _bank_of(m, kind, fh)
                    fb = 0 if fh == 0 else FH
                    out = accs[bk][:cols, 4 * (a - fb) : 4 * (b + 1 - fb)]
                    nc.tensor.matmul(
                        out, lhsT, rhs, start=False, stop=(_LAST[bk] == ei)
                    )
                if ei == _GSPL[2] - 1:      # end of m1 pass
                    flush([4, 5, 6, 7], [nc.vector, nc.scalar])
                if ei == _A_HI_END - 1:     # end of m0 hi entries
                    flush([2, 3], [nc.vector, nc.scalar])
            flush([0, 1], [nc.vector, nc.scalar])
    nc.finalize()
    _dedupe_ldweights(nc)
    return nc


def _dedupe_ldweights(nc):
    """Drop back-to-back InstLdweights with identical weights APs."""
    for fn in nc.m.functions:
        for bb in fn.blocks:
            insts = list(bb.instructions)
            keep = []
            prev_key = None
            for inst in insts:
                if type(inst).__name__ == 'InstLdweights':
                    key = str(inst.ins[0])
                    si = inst.sync_info
                    clean = not si or (
                        len(si.on_wait) == 0 and len(si.on_update) == 0
                    )
                    if key == prev_key and clean:
                        continue
                    prev_key = key
                keep.append(inst)
            if len(keep) != len(insts):
                bb.instructions = keep


def _pack_inputs(x, kr, ki):
    import ml_dtypes

    bf16 = ml_dtypes.bfloat16
    xf = np.ascontiguousarray(
        np.asarray(x, dtype=np.float32).reshape(NTRACKS, SR_T)
    )
    kr = np.asarray(kr, dtype=np.float32)
    ki = np.asarray(ki, dtype=np.float32)

    xpad = np.zeros((NTRACKS, XPAD_CH * PCH), np.float32)
    xpad[:, L // 2 : L // 2 + SR_T] = xf
    xch = xpad.reshape(NTRACKS, XPAD_CH, PCH)      # [t, j, p]

    in_maps = []
    for q in range(NCORES):
        ja = q + 4 * (VA0 + np.arange(VA))
        jb = (q + 3) + 4 * (VB0 + np.arange(VB))
        xa = np.ascontiguousarray(
            xch[:, ja, :].transpose(2, 1, 0).astype(bf16)
        )  # [128, VA, 4]
        xb = np.ascontiguousarray(
            xch[:, jb, :].transpose(2, 1, 0).astype(bf16)
        )
        kt = np.zeros((128, int(_KTOFF[-1])), np.float32)
        for ei, (m, s, kind, cols, n0, n1) in enumerate(_ENTRIES):
            c = (M1C0 + 8 * s + q) if m == 1 else (8 * s + q)
            if m == 0 and c >= NCH:
                continue  # chunks 542/543 on cores 6-7: zero weights
            binoff = (128 if m == 1 else 0) + (64 if kind == 1 else 0)
            nb = cols // 2
            off = int(_KTOFF[ei])
            blk = kt[:, off : off + cols]
            sl = slice(c * PCH, (c + 1) * PCH)
            blk[:, 0::2] = kr[binoff : binoff + nb, sl].T
            blk[:, 1::2] = ki[binoff : binoff + nb, sl].T
        ktb = kt.astype(bf16)
        im = {
            f"kt{g}": np.ascontiguousarray(
                ktb[:, int(_KTOFF[_GSPL[g]]) : int(_KTOFF[_GSPL[g + 1]])]
            )
            for g in range(_NG)
        }
        im["xa"] = xa
        im["xb"] = xb
        in_maps.append(im)
    return in_maps


def _combine(outs):
    boff = np.cumsum([0] + _BANK_COLS)
    re_acc = np.zeros((KBINS, NF, NTRACKS), np.float32)
    im_acc = np.zeros((KBINS, NF, NTRACKS), np.float32)
    # bank -> (bin base, frame base)
    meta = {0: (0, 0), 1: (0, FH), 2: (64, 0), 3: (64, FH),
            4: (128, 0), 5: (128, FH), 6: (192, 0), 7: (192, FH)}
    for q in range(NCORES):
        o = np.asarray(outs[q]).astype(np.float32)
        for b in range(8):
            kb, fb = meta[b]
            nfr = (_BANK_COLS[b]) // 4
            nbins = min(64, KBINS - kb)
            blk = o[: 2 * nbins, int(boff[b]) : int(boff[b + 1])]
            blk = blk.reshape(2 * nbins, nfr, 4)
            re_acc[kb : kb + nbins, fb : fb + nfr] += blk[0::2]
            im_acc[kb : kb + nbins, fb : fb + nfr] += blk[1::2]
    y = np.sqrt(re_acc**2 + im_acc**2)  # [252, 129, 4]
    # output (B, K, NF, Tr): track t = b*SR_TR + tr
    y = y.reshape(KBINS, NF, SR_B, SR_TR)
    return np.ascontiguousarray(y.transpose(2, 0, 1, 3))


def kernel(x, kr, ki):
    global _PROG
    from concourse.bass_utils import run_bass_kernel_spmd

    if _PROG is None:
        _PROG = _build_program()
    in_maps = _pack_inputs(x, kr, ki)
    res = run_bass_kernel_spmd(_PROG, in_maps, core_ids=list(range(NCORES)))
    outs = [res.results[q]["out"] for q in range(NCORES)]
    return _combine(outs)


# revision 14
# speedup vs baseline: 2.0211x; 1.0100x over previous
"""CQT magnitude kernel for Trainium2 (8 NeuronCores, Bass/Tile).

Strategy (v3)
-------------
C[k, n] = sum_l xpad[n*HOP + l] * kernel[k, l], tiled over 128-wide
l-chunks.  Core q owns chunks c = 8s+q (m0 = bins 0-127) and
c = 247+8s+q (m1 = bins 128-251); host sums the 8 per-core partials.

Key points vs v2 (74.8us -> target ~31us):
 * Contiguous rhs streams.  Core q only ever touches xpad chunks of a
   single residue class mod 4 (j = q mod 4 for m0, q+3 mod 4 for m1), so
   xi is packed per-class as [128, v, 4 tracks] with v = signal chunk/4.
   The matmul rhs walk [ [4,F], [1,4] ] is then fully sequential in
   SBUF.  Measured: strided rhs streams at 0.90 ns/col, contiguous at
   0.427 ns/col (1 col/cycle @ 2.4 GHz) -- a 2.1x PE speedup.
 * re/im interleaved weight columns: col 2t = kr[bin t], 2t+1 = ki.
   For chunks with M<=64 active bins one matmul computes both parts
   (row 2t = re, 2t+1 = im), halving streamed columns there.  Bins are
   split lo (0-63) / hi (64-127) per half so PSUM row meanings stay
   uniform across chunks; hi entries exist only where M>64.
 * 8 PSUM accumulators: (m0,m1) x (lo,hi) x (frame half 0-64 / 65-128),
   each [128, <=260] f32 (2KB bank limit forces the frame split).
 * Banks are initialized by zero-weight matmuls (start=True, full
   rectangle) during the DMA wait -- they double as PE p-state warmup.
   All real entries accumulate with start=False in any order, so the
   short, LDWEIGHTS-bound edge entries run first while the PE clock
   still ramps.
 * Flush pipelining: m1 pass runs early and its banks flush under the
   long m0 pass; m0-hi banks flush under the m0 lo-only tail; only the
   m0-lo flush remains in the tail.  Output staged to bf16 (halves
   DMA; partial sums re-summed on host in f32, ~0.1% error).
"""

import numpy as np

# ---- problem constants (hardcoded per contract) ----
SR = 44100
BPO = 36
KBINS = 252
FMIN = 32.70319566257483
QF = 1.0 / (2.0 ** (1.0 / BPO) - 1.0)
SR_B, SR_TR, SR_T = 2, 2, 65536
NTRACKS = SR_B * SR_TR            # 4
L = 69376                          # filterbank window length
HOP = 512
PCH = 128
NCH = L // PCH                     # 542 l-chunks
NF = 1 + SR_T // HOP               # 129 frames
NCORES = 8
M1C0 = 247                         # first m1 chunk
NS0 = 68                           # m0 slots per core
NS1 = 6                            # m1 slots per core (48 chunks)
J_VALID_LO, J_VALID_HI = 271, 782  # nonzero xpad chunk range (inclusive)
XPAD_CH = 1056
FH = 65                            # frame-half boundary: fh0=[0,64], fh1=[65,128]
VA0, VA = 64, 134                  # xiA v-window (v = (j - q)/4)
VB0, VB = 64, 133                  # xiB v-window (v = (j - q - 3)/4)
N_PRE = 12                         # zero-weight PE ramp-hold matmuls
S0_INIT = 33                       # m0 init slot (full frame coverage)
S1_INIT = 2                        # m1 init slot (forced full coverage)

# ---- slot tables ----


def _build_tables():
    freqs = FMIN * 2.0 ** (np.arange(KBINS) / BPO)
    lens = QF * SR / freqs
    lo = np.floor((L // 2 - lens / 2) / PCH).astype(int)
    hi = np.ceil((L // 2 + lens / 2) / PCH).astype(int)
    m0c = np.zeros(NCH + 8, int)
    m1c = np.zeros(NCH + 8, int)
    for k in range(128):
        m0c[lo[k] : hi[k]] = np.maximum(m0c[lo[k] : hi[k]], k + 1)
    for k in range(128, KBINS):
        m1c[lo[k] : hi[k]] = np.maximum(m1c[lo[k] : hi[k]], k - 127)
    m0s = [max(m0c[8 * s + q] for q in range(8)) for s in range(NS0)]
    m1s = [max(m1c[M1C0 + 8 * s + q] for q in range(8)) for s in range(NS1)]

    def nrng(cl, ch):
        return max(0, -(-(J_VALID_LO - ch) // 4)), min(
            NF - 1, (J_VALID_HI - cl) // 4
        )

    f0 = [nrng(8 * s, 8 * s + 7) for s in range(NS0)]
    f1 = [nrng(M1C0 + 8 * s, M1C0 + 8 * s + 7) for s in range(NS1)]

    # emission order: INIT (4 full-coverage start=True entries, run during
    # the PE clock ramp), phase E (short-F edge slots, LDWEIGHTS-bound so
    # also ramp-insensitive), phase B (m1), phase A (m0 rest, hi first)
    e_slots = []
    for i in range(9):
        e_slots.append(i)
        e_slots.append(67 - i)
    hi_slots = [s for s in range(NS0) if m0s[s] > 64 and s != S0_INIT]
    a_rest = [s for s in range(9, 59) if s not in hi_slots and s != S0_INIT]
    b_order = [s for s in [3, 1, 4, 0, 5] ]

    # entry: (m, s, kind, cols, n0, n1)   kind 0=lo 1=hi
    entries = [
        (0, S0_INIT, 0, 128, 0, NF - 1),
        (0, S0_INIT, 1, 128, 0, NF - 1),
        (1, S1_INIT, 0, 128, 0, NF - 1),
        (1, S1_INIT, 1, 128, 0, NF - 1),   # cols padded past bin 251
    ]
    n_init = len(entries)
    for s in e_slots:
        entries.append((0, s, 0, 2 * min(m0s[s], 64), f0[s][0], f0[s][1]))
    for s in b_order:
        entries.append((1, s, 0, 2 * min(m1s[s], 64), f1[s][0], f1[s][1]))
        if m1s[s] > 64:
            entries.append((1, s, 1, 2 * (m1s[s] - 64), f1[s][0], f1[s][1]))
    for s in hi_slots:
        entries.append((0, s, 0, 2 * min(m0s[s], 64), f0[s][0], f0[s][1]))
        entries.append((0, s, 1, 2 * (m0s[s] - 64), f0[s][0], f0[s][1]))
    for s in a_rest:
        entries.append((0, s, 0, 2 * min(m0s[s], 64), f0[s][0], f0[s][1]))

    n_e = len(e_slots)
    n_b = sum(1 + (m1s[s] > 64) for s in b_order)
    n_ahi = 2 * len(hi_slots)
    # kt DMA groups: INIT | E | B | A-hi+rest split in 3
    offs = np.cumsum([0] + [e[3] for e in entries])
    e0 = n_init
    b0 = n_init + n_e
    a0 = b0 + n_b
    rest = int(offs[-1] - offs[a0])
    t1 = int(offs[a0]) + rest // 3
    t2 = int(offs[a0]) + 2 * rest // 3
    g2 = int(np.searchsorted(offs, t1))
    g3 = int(np.searchsorted(offs, t2))
    gsplits = [0, e0, b0, a0, g2, g3, len(entries)]
    return entries, offs, gsplits, (n_init, a0, a0 + n_ahi)


_ENTRIES, _KTOFF, _GSPL, (_N_INIT, _B_END, _A_HI_END) = _build_tables()
_NG = len(_GSPL) - 1
_GCOLS = [int(_KTOFF[_GSPL[g + 1]] - _KTOFF[_GSPL[g]]) for g in range(_NG)]

# bank ids: 0=A_lo0 1=A_lo1 2=A_hi0 3=A_hi1 4=B_lo0 5=B_lo1 6=B_hi0 7=B_hi1
_BANK_COLS = [260, 256, 260, 256, 260, 256, 260, 256]


def _bank_of(m, kind, fh):
    return (4 if m == 1 else 0) + 2 * kind + fh


def _fh_windows(n0, n1):
    """Split [n0, n1] at the frame-half boundary; yields (fh, a, b)."""
    out = []
    if n0 < FH:
        out.append((0, n0, min(n1, FH - 1)))
    if n1 >= FH:
        out.append((1, max(n0, FH), n1))
    return out


def _last_writer_per_bank():
    last = {}
    for ei, (m, s, kind, cols, n0, n1) in enumerate(_ENTRIES):
        for fh, a, b in _fh_windows(n0, n1):
            last[_bank_of(m, kind, fh)] = ei
    return last


_LAST = _last_writer_per_bank()

_PROG = None


def _build_program():
    import concourse.bass as bass
    import concourse.mybir as mybir
    from concourse import bacc
    from concourse.tile import TileContext

    f32 = mybir.dt.float32
    bf16 = mybir.dt.bfloat16

    nc = bacc.Bacc(None, name="cqt_spmd3")
    kt_d = [
        nc.dram_tensor(f"kt{g}", [128, _GCOLS[g]], bf16, kind="ExternalInput")
        for g in range(_NG)
    ]
    xa_d = nc.dram_tensor("xa", [128, VA, 4], bf16, kind="ExternalInput")
    xb_d = nc.dram_tensor("xb", [128, VB, 4], bf16, kind="ExternalInput")
    OUTC = sum(_BANK_COLS)
    out_d = nc.dram_tensor("out", [128, OUTC], bf16, kind="ExternalOutput")

    with TileContext(nc) as tc:
        with (
            tc.tile_pool(name="xp", bufs=1) as xp,
            tc.tile_pool(name="ktp", bufs=1) as ktp,
            tc.tile_pool(name="wp", bufs=1) as wp,
            tc.tile_pool(name="accp", bufs=1, space="PSUM") as accp,
        ):
            wtile = wp.tile([128, 128], bf16, tag="wt", name="wt")
            nc.vector.memset(wtile, 0.0)

            xa_t = xp.tile([128, VA, 4], bf16, tag="xa", name="xa")
            xb_t = xp.tile([128, VB, 4], bf16, tag="xb", name="xb")
            kt_t = [
                ktp.tile([128, _GCOLS[g]], bf16, tag=f"kt{g}", name=f"kt{g}")
                for g in range(_NG)
            ]
            # DMA issue split across engines; critical (kt-init, xa, xb) first
            nc.scalar.dma_start(out=kt_t[0], in_=kt_d[0][:, :])
            nc.gpsimd.dma_start(out=xa_t, in_=xa_d[:, :, :])
            nc.scalar.dma_start(out=xb_t, in_=xb_d[:, :, :])
            nc.gpsimd.dma_start(out=kt_t[1], in_=kt_d[1][:, :])
            nc.scalar.dma_start(out=kt_t[2], in_=kt_d[2][:, :])
            nc.gpsimd.dma_start(out=kt_t[3], in_=kt_d[3][:, :])
            nc.gpsimd.dma_start(out=kt_t[4], in_=kt_d[4][:, :])
            nc.gpsimd.dma_start(out=kt_t[5], in_=kt_d[5][:, :])

            # full 2KB banks so no tile ever crosses a PSUM bank boundary
            accs = [
                accp.tile([128, 512], f32, tag=f"acc{b}", name=f"acc{b}")
                for b in range(8)
            ]

            # zero-weight ramp-hold matmuls (no data deps); real INIT
            # entries re-init bank 0 with start=True afterwards
            def zrhs(cols):
                return bass.AP(
                    tensor=wtile.tensor,
                    offset=wtile.offset,
                    ap=[wtile.ap[0], [0, cols // 4], [1, 4]],
                )

            for _ in range(N_PRE):
                nc.tensor.matmul(
                    accs[0][:128, :260], wtile[:, :128], zrhs(260),
                    start=True, stop=True,
                )

            st = wp.tile([128, OUTC], bf16, tag="st", name="st")
            boff = np.cumsum([0] + _BANK_COLS)

            def flush(banks, eng_cycle):
                for i, b in enumerate(banks):
                    eng = eng_cycle[i % len(eng_cycle)]
                    src = accs[b][:128, : _BANK_COLS[b]]
                    dst = st[:, int(boff[b]) : int(boff[b + 1])]
                    if hasattr(eng, "tensor_copy"):
                        eng.tensor_copy(dst, src)
                    else:
                        eng.copy(dst, src)
                lo = int(boff[banks[0]])
                hi = int(boff[banks[-1] + 1])
                nc.gpsimd.dma_start(out=out_d[:, lo:hi], in_=st[:, lo:hi])

            g = 0
            for ei, (m, s, kind, cols, n0, n1) in enumerate(_ENTRIES):
                while ei >= _GSPL[g + 1]:
                    g += 1
                off = int(_KTOFF[ei] - _KTOFF[_GSPL[g]])
                lhsT = kt_t[g][:, off : off + cols]
                xi = xb_t if m == 1 else xa_t
                vbase = (61 + 2 * s - VB0) if m == 1 else (2 * s - VA0)
                for fh, a, b in _fh_windows(n0, n1):
                    F = b - a + 1
                    rhs = bass.AP(
                        tensor=xi.tensor,
                        offset=xi.offset + (vbase + a) * 4,
                        ap=[xi.ap[0], [4, F], [1, 4]],
                    )
                    bk = _bank_of(m, kind, fh)
                    fb = 0 if fh == 0 else FH
                    out = accs[bk][:cols, 4 * (a - fb) : 4 * (b + 1 - fb)]
                    nc.tensor.matmul(
                        out, lhsT, rhs,
                        start=(ei < _N_INIT), stop=(_LAST[bk] == ei),
                    )
                if ei == _B_END - 1:        # end of m1 pass
                    flush([4, 5, 6, 7], [nc.vector, nc.scalar])
                if ei == _A_HI_END - 1:     # end of m0 hi entries
                    flush([2, 3], [nc.vector, nc.scalar])
            # final flush: copies split, out-DMA in 4 pieces on 4 queues
            for i, b in enumerate([0, 1]):
                eng = [nc.vector, nc.scalar][i]
                src = accs[b][:128, : _BANK_COLS[b]]
                dst = st[:, int(boff[b]) : int(boff[b + 1])]
                if hasattr(eng, "tensor_copy"):
                    eng.tensor_copy(dst, src)
                else:
                    eng.copy(dst, src)
            fin = int(boff[2])
            cuts = [0, fin // 3, 2 * fin // 3, fin]
            for i, eng in enumerate([nc.gpsimd, nc.scalar, nc.sync]):
                eng.dma_start(
                    out=out_d[:, cuts[i] : cuts[i + 1]],
                    in_=st[:, cuts[i] : cuts[i + 1]],
                )
    nc.finalize()
    _dedupe_ldweights(nc)
    return nc


def _dedupe_ldweights(nc):
    """Drop back-to-back InstLdweights with identical weights APs."""
    for fn in nc.m.functions:
        for bb in fn.blocks:
            insts = list(bb.instructions)
            keep = []
            prev_key = None
            for inst in insts:
                if type(inst).__name__ == 'InstLdweights':
                    key = str(inst.ins[0])
                    si = inst.sync_info
                    clean = not si or (
                        len(si.on_wait) == 0 and len(si.on_update) == 0
                    )
                    if key == prev_key and clean:
                        continue
                    prev_key = key
                keep.append(inst)
            if len(keep) != len(insts):
                bb.instructions = keep


def _pack_inputs(x, kr, ki):
    import ml_dtypes

    bf16 = ml_dtypes.bfloat16
    xf = np.ascontiguousarray(
        np.asarray(x, dtype=np.float32).reshape(NTRACKS, SR_T)
    )
    kr = np.asarray(kr, dtype=np.float32)
    ki = np.asarray(ki, dtype=np.float32)

    xpad = np.zeros((NTRACKS, XPAD_CH * PCH), np.float32)
    xpad[:, L // 2 : L // 2 + SR_T] = xf
    xch = xpad.reshape(NTRACKS, XPAD_CH, PCH)      # [t, j, p]

    in_maps = []
    for q in range(NCORES):
        ja = q + 4 * (VA0 + np.arange(VA))
        jb = (q + 3) + 4 * (VB0 + np.arange(VB))
        xa = np.ascontiguousarray(
            xch[:, ja, :].transpose(2, 1, 0).astype(bf16)
        )  # [128, VA, 4]
        xb = np.ascontiguousarray(
            xch[:, jb, :].transpose(2, 1, 0).astype(bf16)
        )
        kt = np.zeros((128, int(_KTOFF[-1])), np.float32)
        for ei, (m, s, kind, cols, n0, n1) in enumerate(_ENTRIES):
            c = (M1C0 + 8 * s + q) if m == 1 else (8 * s + q)
            if m == 0 and c >= NCH:
                continue  # chunks 542/543 on cores 6-7: zero weights
            binoff = (128 if m == 1 else 0) + (64 if kind == 1 else 0)
            nb = min(cols // 2, KBINS - binoff)   # init m1-hi pads past 251
            off = int(_KTOFF[ei])
            blk = kt[:, off : off + cols]
            sl = slice(c * PCH, (c + 1) * PCH)
            blk[:, 0 : 2 * nb : 2] = kr[binoff : binoff + nb, sl].T
            blk[:, 1 : 2 * nb : 2] = ki[binoff : binoff + nb, sl].T
        ktb = kt.astype(bf16)
        im = {
            f"kt{g}": np.ascontiguousarray(
                ktb[:, int(_KTOFF[_GSPL[g]]) : int(_KTOFF[_GSPL[g + 1]])]
            )
            for g in range(_NG)
        }
        im["xa"] = xa
        im["xb"] = xb
        in_maps.append(im)
    return in_maps


def _combine(outs):
    boff = np.cumsum([0] + _BANK_COLS)
    re_acc = np.zeros((KBINS, NF, NTRACKS), np.float32)
    im_acc = np.zeros((KBINS, NF, NTRACKS), np.float32)
    # bank -> (bin base, frame base)
    meta = {0: (0, 0), 1: (0, FH), 2: (64, 0), 3: (64, FH),
            4: (128, 0), 5: (128, FH), 6: (192, 0), 7: (192, FH)}
    for q in range(NCORES):
        o = np.asarray(outs[q]).astype(np.float32)
        for b in range(8):
            kb, fb = meta[b]
            nfr = (_BANK_COLS[b]) // 4
            nbins = min(64, KBINS - kb)
            blk = o[: 2 * nbins, int(boff[b]) : int(boff[b + 1])]
            blk = blk.reshape(2 * nbins, nfr, 4)
            re_acc[kb : kb + nbins, fb : fb + nfr] += blk[0::2]
            im_acc[kb : kb + nbins, fb : fb + nfr] += blk[1::2]
    y = np.sqrt(re_acc**2 + im_acc**2)  # [252, 129, 4]
    # output (B, K, NF, Tr): track t = b*SR_TR + tr
    y = y.reshape(KBINS, NF, SR_B, SR_TR)
    return np.ascontiguousarray(y.transpose(2, 0, 1, 3))


def kernel(x, kr, ki):
    global _PROG
    from concourse.bass_utils import run_bass_kernel_spmd

    if _PROG is None:
        _PROG = _build_program()
    in_maps = _pack_inputs(x, kr, ki)
    res = run_bass_kernel_spmd(_PROG, in_maps, core_ids=list(range(NCORES)))
    outs = [res.results[q]["out"] for q in range(NCORES)]
    return _combine(outs)
